# revision 26
# baseline (speedup 1.0000x reference)
"""Trainium2 Bass kernel for nn_DockingTimeModel (2-layer GINE GNN + mean-pool
+ MLP head), single merged SPMD launch on 8 NeuronCores.

v2 design: data-parallel over graphs (core c owns graphs [512c, 512(c+1)) and
their contiguous node range; edges live on the dst-owner core). Edges are
host-sorted by dst and packed into 128-edge chunks grouped per 128-node tile
(chunk counts maxed over cores so the module is SPMD-uniform). Per chunk:
dma_gather x[src] rows (256B) from compacted int16-indexed tables, edge linear
on PE (bias folded via ones-row, bf16), msg = relu(xg + lin) -> bf16, and the
segment-sum runs ON PE: aggT[64f, 128n] += msg[128e,64f].T @ S[128e,128n]
where S = is_equal(dstrel, iota) is built on DVE. No scatter DMA, no HBM
accumulator, aggregate lands feat-major in PSUM. Node MLPs consume it
directly; h1 is produced both feat-major (SBUF-resident bf16 for layer 2) and
node-major (DRAM for the AllToAll pre-gather). One AllToAll exchanges the h1
rows each core needs. Mean-pool via per-tile indicator matmuls; b22 is added
post-mean (linear). Head MLP on-chip; output [1, 512] per core.
"""
import sys

sys.path.insert(0, "/opt/trn_rl_repo")

import math
from contextlib import ExitStack
from dataclasses import dataclass, field

import numpy as np
from ml_dtypes import bfloat16

from concourse import bacc, bass, mybir, tile
from concourse import bass_utils
from concourse.masks import make_identity

F32 = mybir.dt.float32
BF16 = mybir.dt.bfloat16
I16 = mybir.dt.int16
AF = mybir.ActivationFunctionType
ALU = mybir.AluOpType

C = 8
P = 128
ND = 64
ED = 16
EMB = 128
USR = 12
EAROWS = ED + 1          # edge-attr rows + ones row (bias)
TAB0 = 32768
NCH = 512                # node-MLP group size
CPD = 16                 # gather chunks per DMA window


def _wrap16(idx):
    L = len(idx)
    assert L % 16 == 0
    a = np.asarray(idx, np.int16).reshape(L // 16, 16).T
    return np.tile(a, (8, 1))


@dataclass
class Layer:
    """Static chunk structure for one layer's edge phase (uniform across
    cores). k[t, s] chunks of 128 edges for node-tile t from stream s
    (s = table id). pos[t, s] = chunk offset within stream s."""
    k: np.ndarray                 # [NT, 2]
    pos: np.ndarray               # [NT, 2]
    nch: tuple                    # chunks per stream
    TAB1: int = 0                 # rows in table 1

    @property
    def slots(self):
        return (self.nch[0] * P, self.nch[1] * P)


@dataclass
class CFG:
    TAB1: int
    B: int
    N_SH: int
    L1: Layer
    L2: Layer
    B1: int = 0
    B2: int = 0
    H: int = 0
    GS: int = 512
    GSP: int = 512
    GW: int = 8
    PG: int = 2
    n_pool_idx: int = 0

    @property
    def NT(self):
        return self.N_SH // P


def _mk_layer(tb_all, tile_all, NT):
    """Uniform chunk structure: k[t, s] = max over cores of
    ceil(count(tile==t, tb==s) / 128)."""
    k = np.zeros((NT, 2), np.int64)
    for tb, tl in zip(tb_all, tile_all):
        for s in (0, 1):
            cnt = np.bincount(tl[tb == s], minlength=NT)
            k[:, s] = np.maximum(k[:, s], (cnt + P - 1) // P)
    dead = k.sum(1) == 0
    k[dead, 0] = 1
    pos = np.zeros((NT, 2), np.int64)
    pos[:, 0] = np.cumsum(k[:, 0]) - k[:, 0]
    pos[:, 1] = np.cumsum(k[:, 1]) - k[:, 1]
    return Layer(k=k, pos=pos, nch=(int(k[:, 0].sum()), int(k[:, 1].sum())))


def _fill_layer(lay, tb, tl, idx_local, dloc, ea, NT):
    """Place one core's edges into the two slot streams. Returns per stream:
    (gidx int16, eaT bf16 [EAROWS, slots], dstrel f32 [slots])."""
    out = []
    for s in (0, 1):
        slots = lay.nch[s] * P
        gidx = np.zeros(slots, np.int16)
        eaT = np.zeros((EAROWS, slots), np.float32)
        dstrel = np.full(slots, 255.0, np.float32)
        sel = np.nonzero(tb == s)[0]
        if len(sel):
            tls = tl[sel]
            order = sel[np.argsort(tls, kind="stable")]
            tls = tl[order]
            starts = np.searchsorted(tls, np.arange(NT))
            ends = np.searchsorted(tls, np.arange(NT), side="right")
            for t in range(NT):
                e = order[starts[t]:ends[t]]
                if not len(e):
                    continue
                o0 = lay.pos[t, s] * P
                assert len(e) <= lay.k[t, s] * P
                gidx[o0:o0 + len(e)] = idx_local[e].astype(np.int16)
                eaT[:ED, o0:o0 + len(e)] = ea[e].T
                eaT[ED, o0:o0 + len(e)] = 1.0
                dstrel[o0:o0 + len(e)] = (dloc[e] - t * P).astype(np.float32)
        out.append((gidx, eaT.astype(bfloat16),
                    dstrel.reshape(-1, P).T.astype(bfloat16)))
    return out


def _preprocess(x, edge_index, edge_attr, batch, G=4096):
    src = np.asarray(edge_index[0], np.int64)
    dst = np.asarray(edge_index[1], np.int64)
    batch = np.asarray(batch, np.int64)
    ea = np.asarray(edge_attr, np.float32)
    GS = G // C
    gb = np.searchsorted(batch, np.arange(0, G + 1, GS))
    ncnt = np.diff(gb)
    N_SH = int(math.ceil(ncnt.max() / NCH) * NCH)
    NT = N_SH // P
    owner = np.searchsorted(gb, dst, side="right") - 1

    cores = []
    for c in range(C):
        em = np.nonzero(owner == c)[0]
        s_c, d_c = src[em], dst[em] - gb[c]
        o = np.argsort(d_c, kind="stable")
        s_c, d_c, ea_c = s_c[o], d_c[o], ea[em[o]]
        uniq, inv = np.unique(s_c, return_inverse=True)
        cores.append(dict(s=s_c, dloc=d_c, ea=ea_c, uniq=uniq, inv=inv,
                          tile=d_c // P, n_c=ncnt[c]))
    max_u = max(len(pc["uniq"]) for pc in cores)
    TAB1 = int(math.ceil(max(max_u - TAB0, P) / P) * P)

    # layer-1 structure: table split by uniq position
    tb1 = [(pc["inv"] >= TAB0).astype(np.int64) for pc in cores]
    L1 = _mk_layer(tb1, [pc["tile"] for pc in cores], NT)
    L1.TAB1 = TAB1

    # a2a in two phases split at owner-local node id H: phase tables are
    # each a single <=32768-row gather table and collective #1 overlaps the
    # tail of layer 1.
    for c in range(C):
        own = np.searchsorted(gb, cores[c]["uniq"], side="right") - 1
        cores[c]["uniq_owner"] = own
        cores[c]["uniq_lid"] = cores[c]["uniq"] - gb[own]
    H = (int(N_SH * 0.75) // NCH) * NCH
    while H > 0:
        need1 = np.zeros((C, C), np.int64)
        for c in range(C):
            own, lid = cores[c]["uniq_owner"], cores[c]["uniq_lid"]
            for o in range(C):
                need1[c, o] = int(((own == o) & (lid < H)).sum())
        B1 = int(math.ceil((need1.max() + 1) / P) * P)
        if C * B1 <= TAB0:
            break
        H -= NCH
    assert H > 0
    need2 = np.zeros((C, C), np.int64)
    for c in range(C):
        own, lid = cores[c]["uniq_owner"], cores[c]["uniq_lid"]
        for o in range(C):
            need2[c, o] = int(((own == o) & (lid >= H)).sum())
    B2 = int(math.ceil((need2.max() + 1) / P) * P)
    assert C * B2 <= TAB0
    for c in range(C):
        own, lid = cores[c]["uniq_owner"], cores[c]["uniq_lid"]
        ph = (lid >= H).astype(np.int64)
        r = np.zeros(len(own), np.int64)
        for o in range(C):
            for p_ in (0, 1):
                m = (own == o) & (ph == p_)
                r[m] = np.arange(m.sum())
        posu = np.where(ph == 0, own * B1 + r, own * B2 + r)
        cores[c]["pos2"] = posu[cores[c]["inv"]]
        cores[c]["ph2"] = ph[cores[c]["inv"]]

    tb2 = [pc["ph2"] for pc in cores]
    L2 = _mk_layer(tb2, [pc["tile"] for pc in cores], NT)
    B = B1

    GSP = GS
    cfg = CFG(TAB1=TAB1, B=B, N_SH=N_SH, L1=L1, L2=L2, GS=GS, GSP=GSP,
              B1=B1, B2=B2, H=H)

    per_core = []
    for c in range(C):
        pc = cores[c]
        g1 = _fill_layer(L1, tb1[c], pc["tile"], np.where(
            pc["inv"] < TAB0, pc["inv"], pc["inv"] - TAB0),
            pc["dloc"], pc["ea"], NT)
        g2 = _fill_layer(L2, tb2[c], pc["tile"], pc["pos2"],
                         pc["dloc"], pc["ea"], NT)

        n_c = pc["n_c"]
        xT = np.zeros((ND, N_SH), np.float32)
        xT[:, :n_c] = np.asarray(x)[gb[c]:gb[c + 1]].T

        # pooling structures (same scheme as v1)
        bl = batch[gb[c]:gb[c + 1]] - c * GS
        blp = np.full(N_SH, -1, np.int64)
        blp[:n_c] = bl
        tiles = blp.reshape(NT, P)
        g_first = np.array([t[t >= 0].min() if (t >= 0).any() else 0
                            for t in tiles])
        relg = np.where(blp >= 0, blp - np.repeat(g_first, P), 255.0)
        cnt = np.bincount(bl, minlength=GS).astype(np.float32)
        gstart = np.searchsorted(bl, np.arange(GS))
        gend = np.searchsorted(bl, np.arange(GS), side="right")
        t_lo, t_hi = gstart // P, np.maximum(gend - 1, gstart) // P

        per_core.append(dict(
            g1=g1, g2=g2, xT=xT, uniq=pc["uniq"],
            uniq_owner=pc["uniq_owner"], n_c=n_c,
            relg=relg.astype(np.float32), g_first=g_first, cnt=cnt,
            t_lo=t_lo, t_hi=t_hi))

    # send-side gather indices, per phase (phase 2 rows offset by -H
    # because they gather from h1r2)
    for o in range(C):
        sg1 = np.zeros(C * B1, np.int16)
        sg2 = np.zeros(C * B2, np.int16)
        for c in range(C):
            m = per_core[c]["uniq_owner"] == o
            rows = per_core[c]["uniq"][m] - gb[o]
            r1 = rows[rows < H]
            r2 = rows[rows >= H] - H
            sg1[c * B1:c * B1 + len(r1)] = r1.astype(np.int16)
            sg2[c * B2:c * B2 + len(r2)] = r2.astype(np.int16)
        per_core[o]["sg1"] = sg1
        per_core[o]["sg2"] = sg2

    cfg.GW = int(max((pc["relg"][pc["relg"] != 255.0]).max() + 1
                     if (pc["relg"] != 255.0).any() else 1 for pc in per_core))
    cfg.PG = int(max((pc["t_hi"] - pc["t_lo"] + 1)[pc["cnt"] > 0].max()
                     if (pc["cnt"] > 0).any() else 1 for pc in per_core))
    cfg.n_pool_idx = int(math.ceil(cfg.PG * cfg.GSP / P) * P)

    ZPAD = NT * cfg.GW
    for pc in per_core:
        pidx = np.full(cfg.n_pool_idx, ZPAD, np.int16)
        for g in range(GS):
            if pc["cnt"][g] <= 0:
                continue
            for p_, t in enumerate(range(pc["t_lo"][g], pc["t_hi"][g] + 1)):
                rel = g - pc["g_first"][t]
                pidx[p_ * cfg.GSP + g] = t * cfg.GW + rel
        pc["pool_idx"] = pidx
        pc["cnt_gm"] = np.maximum(
            np.pad(pc["cnt"], (0, cfg.GSP - GS)), 1.0
        ).reshape(cfg.GSP // P, P).T.astype(np.float32)

    relids = np.tile(np.arange(cfg.GW, dtype=np.float32), (P, 1))
    return cfg, gb, per_core, relids


def _gather_tables(cfg, per_core, x):
    out = []
    for pc in per_core:
        uniq = pc["uniq"]
        t0 = np.zeros((TAB0, ND), np.float32)
        t1 = np.zeros((cfg.TAB1, ND), np.float32)
        n0 = min(len(uniq), TAB0)
        t0[:n0] = x[uniq[:n0]]
        if len(uniq) > TAB0:
            t1[:len(uniq) - TAB0] = x[uniq[TAB0:]]
        out.append((t0, t1))
    return out


def _edge_node_phases(ctx, tc, nc, cfg, lay, d, tabs, names, w_e, consts,
                      node_cb):
    """Edge phase (gather + lin + msg + S-matmul aggregation) interleaved with
    per-512-node-group node_cb(g, agg_tiles[4])."""
    iota = consts["iota"]
    NT = cfg.NT

    gp = ctx.enter_context(tc.tile_pool(name=f"g{names[0]}", bufs=1))
    gidx = [gp.tile([P, lay.nch[s] * 8], I16, name=f"gx{names[0]}{s}")
            for s in (0, 1)]
    drl = [gp.tile([P, lay.nch[s]], BF16, name=f"dr{names[0]}{s}")
           for s in (0, 1)]
    for s in (0, 1):
        nc.sync.dma_start(gidx[s][:], d[f"gidx{names[0]}{s}"])
        nc.sync.dma_start(drl[s][:], d[f"dstrel{names[0]}{s}"])

    SB = 8                       # lin chunks per PSUM bank
    xp = ctx.enter_context(tc.tile_pool(name=f"xg{names[0]}", bufs=4))
    ep = ctx.enter_context(tc.tile_pool(name=f"ea{names[0]}", bufs=4))
    mp = ctx.enter_context(tc.tile_pool(name=f"ms{names[0]}", bufs=3))
    sp = ctx.enter_context(tc.tile_pool(name=f"S{names[0]}", bufs=3))
    lp = ctx.enter_context(tc.tile_pool(name=f"lin{names[0]}", bufs=2,
                                        space="PSUM"))
    ap = ctx.enter_context(tc.tile_pool(name=f"agg{names[0]}", bufs=1,
                                        space="PSUM"))

    state = [dict(win=-1), dict(win=-1)]

    def window(s, j):
        w = j // CPD
        st = state[s]
        if st["win"] != w:
            n = min(CPD, lay.nch[s] - w * CPD)
            xg = xp.tile([P, CPD * ND], F32, tag=f"xg{s}")
            nc.gpsimd.dma_gather(
                out_ap=xg[:, :n * ND].rearrange("p (k e) -> p k e", e=ND),
                in_ap=tabs[s], idxs_ap=gidx[s][:, w * CPD * 8:(w * CPD + n) * 8],
                num_idxs=n * P, num_idxs_reg=n * P, elem_size=ND,
                single_packet=False)
            eat = ep.tile([EAROWS, CPD * P], BF16, tag=f"ea{s}")
            nc.sync.dma_start(
                eat[:, :n * P],
                d[f"eaT{names[0]}{s}"][:, w * CPD * P:(w * CPD + n) * P])
            # edge linear into PSUM banks of SB chunks, then one add+relu
            # per bank -> bf16 msg; one is_equal for the whole window's S
            msg = mp.tile([P, CPD * ND], BF16, tag=f"ms{s}")
            for b0 in range(0, n, SB):
                nb = min(SB, n - b0)
                linb = lp.tile([P, SB * ND], F32, tag=f"lb{s}")
                for jj in range(nb):
                    nc.tensor.matmul(
                        out=linb[:, jj * ND:(jj + 1) * ND],
                        lhsT=eat[:, (b0 + jj) * P:(b0 + jj + 1) * P],
                        rhs=w_e[:], start=True, stop=True)
                sl = slice(b0 * ND, (b0 + nb) * ND)
                nc.vector.tensor_add(out=msg[:, sl], in0=xg[:, sl],
                                     in1=linb[:, :nb * ND])
                nc.scalar.activation(out=msg[:, sl], in_=msg[:, sl],
                                     func=AF.Relu)
            S = sp.tile([P, CPD * P], BF16, tag=f"S{s}")
            nc.vector.tensor_tensor(
                out=S[:, :n * P].rearrange("p (j c) -> p j c", c=P),
                in0=drl[s][:, w * CPD:w * CPD + n]
                .unsqueeze(2).broadcast_to([P, n, P]),
                in1=iota[:].unsqueeze(1).broadcast_to([P, n, P]),
                op=ALU.is_equal)
            st.update(win=w, msg=msg, S=S)
        return st["msg"], st["S"], j - w * CPD

    agg = None
    for t in range(NT):
        if t % 4 == 0:
            agg = ap.tile([ND, 4 * P], F32, tag=f"agg{(t // 4) % 2}")
        i4 = t % 4
        nk = int(lay.k[t, 0] + lay.k[t, 1])
        ci = 0
        for s in (0, 1):
            for i in range(int(lay.k[t, s])):
                j = int(lay.pos[t, s]) + i
                msg, S, jj = window(s, j)
                nc.tensor.matmul(out=agg[:, i4 * P:(i4 + 1) * P],
                                 lhsT=msg[:, jj * ND:(jj + 1) * ND],
                                 rhs=S[:, jj * P:(jj + 1) * P],
                                 start=(ci == 0), stop=(ci == nk - 1))
                ci += 1
        if t % 4 == 3:
            node_cb(t // 4, agg)


def _build(cfg):
    nc = bacc.Bacc("TRN2", target_bir_lowering=False, debug=False,
                   num_devices=C)
    d = {}

    def inp(name, shape, dt=F32):
        d[name] = nc.dram_tensor(name, shape, dt, kind="ExternalInput").ap()

    inp("tab0", [TAB0, ND]); inp("tab1", [cfg.TAB1, ND])
    for ln, lay in (("1", cfg.L1), ("2", cfg.L2)):
        for s in (0, 1):
            inp(f"gidx{ln}{s}", [P, lay.nch[s] * 8], I16)
            inp(f"eaT{ln}{s}", [EAROWS, lay.nch[s] * P], BF16)
            inp(f"dstrel{ln}{s}", [P, lay.nch[s]], BF16)
    inp("xT", [ND, cfg.N_SH])
    inp("sg1", [P, C * cfg.B1 // 16], I16)
    inp("sg2", [P, C * cfg.B2 // 16], I16)
    inp("w_e1", [EAROWS, ND], BF16); inp("w_e2", [EAROWS, ND], BF16)
    inp("w11", [ND, ND], BF16); inp("b11", [ND, 1])
    inp("w12", [ND, ND], BF16); inp("b12", [ND, 1])
    inp("w12b", [ND + 1, ND], BF16)
    inp("w21", [ND, EMB], BF16); inp("b21", [EMB, 1])
    inp("w22", [EMB, EMB], BF16); inp("b22", [EMB, 1])
    inp("iota", [P, P], BF16)
    inp("relg", [P, cfg.NT], BF16); inp("relids", [P, cfg.GW], BF16)
    inp("pool_idx", [P, cfg.n_pool_idx // 16], I16)
    inp("cnt_gm", [P, cfg.GSP // P]); inp("usrT", [USR, cfg.GSP], BF16)
    for nm, shp in (("hw1a", [EMB, 128]), ("hw1b", [USR, 128]), ("hb1", [128, 1]),
                    ("hw2", [128, 64]), ("hb2", [64, 1]), ("hw3", [64, 32]),
                    ("hb3", [32, 1]), ("hw4", [32, 16]), ("hb4", [16, 1]),
                    ("hw5", [16, 1]), ("hb5", [1, 1])):
        inp(nm, shp, F32 if nm.startswith("hb") else BF16)
    yT = nc.dram_tensor("yT", [1, cfg.GSP], F32, kind="ExternalOutput").ap()

    GW, PG, NT, GSP, B = cfg.GW, cfg.PG, cfg.NT, cfg.GSP, cfg.B
    NROW = NT * GW + P
    NG = cfg.N_SH // NCH

    with tile.TileContext(nc) as tc, ExitStack() as ctx:
        const = ctx.enter_context(tc.tile_pool(name="const", bufs=1))

        def ld(name, shape, dt=F32):
            t = const.tile(shape, dt, name=f"c_{name}")
            nc.sync.dma_start(t[:], d[name])
            return t

        w_e1 = ld("w_e1", [EAROWS, ND], BF16)
        w_e2 = ld("w_e2", [EAROWS, ND], BF16)
        w11 = ld("w11", [ND, ND], BF16); b11 = ld("b11", [ND, 1])
        w12 = ld("w12", [ND, ND], BF16); b12 = ld("b12", [ND, 1])
        w12b = ld("w12b", [ND + 1, ND], BF16)
        w21 = ld("w21", [ND, EMB], BF16); b21 = ld("b21", [EMB, 1])
        w22 = ld("w22", [EMB, EMB], BF16); b22 = ld("b22", [EMB, 1])
        iota = ld("iota", [P, P], BF16)
        relg = ld("relg", [P, NT], BF16)
        relids = ld("relids", [P, GW], BF16)
        ident = const.tile([P, P], F32, name="ident")
        make_identity(nc, ident[:])
        zt = const.tile([P, P], F32, name="zt")
        nc.vector.memset(zt[:], 0.0)
        h1T = const.tile([ND, cfg.N_SH], BF16, name="h1T")

        dram = ctx.enter_context(tc.tile_pool(name="dram", bufs=1, space="DRAM"))
        B1, B2, H = cfg.B1, cfg.B2, cfg.H
        GH = H // NCH
        h1r1 = dram.tile([H, ND], F32)
        h1r2 = dram.tile([cfg.N_SH - H, ND], F32)
        a2a_in1 = dram.tile([C * B1, ND], F32)
        a2a_out1 = dram.tile([C * B1, ND], F32)
        a2a_in2 = dram.tile([C * B2, ND], F32)
        a2a_out2 = dram.tile([C * B2, ND], F32)
        parts = dram.tile([NROW, P], F32)
        nc.sync.dma_start(
            out=parts[NT * GW:NT * GW + P, :].rearrange("(p r) e -> p (r e)", p=P),
            in_=zt[:, :P])

        consts = dict(iota=iota)

        agp = ctx.enter_context(tc.tile_pool(name="a2a", bufs=2))
        sgp = ctx.enter_context(tc.tile_pool(name="a2as", bufs=2))

        def a2a_phase(src_t, sgname, Bp, ain, aout):
            for dest in range(C):
                six = sgp.tile([P, Bp // 16], I16, tag="six")
                nc.sync.dma_start(
                    six[:], d[sgname][:, dest * Bp // 16:(dest + 1) * Bp // 16])
                gt = agp.tile([P, Bp // P * ND], F32, tag="gt")
                nc.gpsimd.dma_gather(
                    out_ap=gt[:].rearrange("p (k e) -> p k e", e=ND),
                    in_ap=src_t[:], idxs_ap=six[:],
                    num_idxs=Bp, num_idxs_reg=Bp, elem_size=ND,
                    single_packet=False)
                nc.sync.dma_start(
                    out=ain[dest * Bp:(dest + 1) * Bp, :]
                    .rearrange("(k p) e -> p k e", p=P),
                    in_=gt[:].rearrange("p (k e) -> p k e", e=ND))
            nc.gpsimd.collective_compute(
                "AllToAll", mybir.AluOpType.bypass,
                replica_groups=[list(range(C))],
                ins=[ain[:].opt()], outs=[aout[:].opt()])

        # ---------------- layer 1 ----------------
        n1ctx = ExitStack()
        xtp = n1ctx.enter_context(tc.tile_pool(name="n1_xt", bufs=2))
        htp = n1ctx.enter_context(tc.tile_pool(name="n1_ht", bufs=2))
        z1p = n1ctx.enter_context(tc.tile_pool(name="n1_z1", bufs=2))
        rwp = n1ctx.enter_context(tc.tile_pool(name="n1_rw", bufs=3))
        nps = n1ctx.enter_context(tc.tile_pool(name="n1_ps", bufs=1,
                                               space="PSUM"))

        def node1(g, agg):
            sl = slice(g * NCH, (g + 1) * NCH)
            xTb = xtp.tile([ND, NCH], F32, tag="xt")
            nc.sync.dma_start(xTb[:], d["xT"][:, sl])
            hT = htp.tile([ND, NCH], BF16, tag="ht")
            nc.vector.tensor_add(out=hT[:], in0=xTb[:], in1=agg[:])
            z1ps = nps.tile([ND, NCH], F32, tag="nps")
            nc.tensor.matmul(out=z1ps[:], lhsT=w11[:], rhs=hT[:],
                             start=True, stop=True)
            z1x = z1p.tile([ND + 1, NCH], BF16, tag="z1")
            nc.scalar.activation(out=z1x[:ND, :], in_=z1ps[:], func=AF.Relu,
                                 bias=b11[:])
            nc.vector.memset(z1x[ND:ND + 1, :], 1.0)
            hps = nps.tile([ND, NCH], F32, tag="nps")
            nc.tensor.matmul(out=hps[:], lhsT=w12[:], rhs=z1x[:ND, :],
                             start=True, stop=True)
            nc.scalar.activation(out=h1T[:, sl], in_=hps[:], func=AF.Relu,
                                 bias=b12[:])
            nmps = nps.tile([P, 4 * ND], F32, tag="nm")
            for i in range(4):
                nc.tensor.matmul(out=nmps[:, i * ND:(i + 1) * ND],
                                 lhsT=z1x[:, i * P:(i + 1) * P],
                                 rhs=w12b[:], start=True, stop=True)
            rows = rwp.tile([P, 4 * ND], F32, tag="rw")
            nc.scalar.activation(out=rows[:], in_=nmps[:], func=AF.Relu)
            h1r_t, r0 = (h1r1, g * NCH) if g < GH else (h1r2, (g - GH) * NCH)
            nc.sync.dma_start(
                out=h1r_t[r0:r0 + NCH, :]
                .rearrange("(i p) e -> p i e", p=P),
                in_=rows[:].rearrange("p (i e) -> p i e", e=ND))
            if g == GH - 1:
                a2a_phase(h1r1, "sg1", B1, a2a_in1, a2a_out1)

        with ExitStack() as ectx:
            _edge_node_phases(ectx, tc, nc, cfg, cfg.L1, d,
                              (d["tab0"], d["tab1"]), "1", w_e1, consts, node1)
        n1ctx.close()

        # ---------------- exchange phase 2 ----------------
        a2a_phase(h1r2, "sg2", B2, a2a_in2, a2a_out2)

        # ---------------- layer 2 ----------------
        t2_0 = a2a_out1[:]
        t2_1 = a2a_out2[:]

        n2ctx = ExitStack()
        h1fp = n2ctx.enter_context(tc.tile_pool(name="n2_h1", bufs=2))
        ht2p = n2ctx.enter_context(tc.tile_pool(name="n2_ht", bufs=2))
        zxp = n2ctx.enter_context(tc.tile_pool(name="n2_zx", bufs=2))
        emp = n2ctx.enter_context(tc.tile_pool(name="n2_em", bufs=3))
        s4p = n2ctx.enter_context(tc.tile_pool(name="n2_s4", bufs=2))
        psp = n2ctx.enter_context(tc.tile_pool(name="n2_pl", bufs=3))
        nps2 = n2ctx.enter_context(tc.tile_pool(name="n2_ps", bufs=1,
                                                space="PSUM"))

        def node2(g, agg):
            sl = slice(g * NCH, (g + 1) * NCH)
            h1f = h1fp.tile([ND, NCH], F32, tag="h1f")
            nc.vector.tensor_copy(out=h1f[:], in_=h1T[:, sl])
            hT2 = ht2p.tile([ND, NCH], BF16, tag="ht2")
            nc.vector.tensor_add(out=hT2[:], in0=h1f[:], in1=agg[:])
            zps = nps2.tile([EMB, NCH], F32, tag="nps")
            nc.tensor.matmul(out=zps[:], lhsT=w21[:], rhs=hT2[:],
                             start=True, stop=True)
            zx = zxp.tile([EMB, NCH], BF16, tag="zx")
            nc.scalar.activation(out=zx[:], in_=zps[:], func=AF.Relu,
                                 bias=b21[:])
            S4 = s4p.tile([P, 4 * GW], BF16, tag="S4")
            nc.vector.tensor_tensor(
                out=S4[:].rearrange("p (j g) -> p j g", g=GW),
                in0=relg[:, g * 4:(g + 1) * 4]
                .unsqueeze(2).broadcast_to([P, 4, GW]),
                in1=relids[:].unsqueeze(1).broadcast_to([P, 4, GW]),
                op=ALU.is_equal)
            emps = nps2.tile([P, 4 * EMB], F32, tag="nm")
            for i in range(4):
                nc.tensor.matmul(out=emps[:, i * EMB:(i + 1) * EMB],
                                 lhsT=zx[:, i * P:(i + 1) * P],
                                 rhs=w22[:], start=True, stop=True)
            emb_nm = emp.tile([P, 4 * EMB], BF16, tag="em")
            nc.vector.tensor_copy(out=emb_nm[:], in_=emps[:])
            pps = nps2.tile([GW, 4 * P], F32, tag="nm")
            for i in range(4):
                nc.tensor.matmul(out=pps[:, i * P:(i + 1) * P],
                                 lhsT=S4[:, i * GW:(i + 1) * GW],
                                 rhs=emb_nm[:, i * EMB:(i + 1) * EMB],
                                 start=True, stop=True)
            psb = psp.tile([GW, 4 * P], F32, tag="pls")
            nc.vector.tensor_copy(out=psb[:], in_=pps[:])
            nc.sync.dma_start(
                out=parts[g * 4 * GW:(g + 1) * 4 * GW, :]
                .rearrange("(i g_) e -> g_ i e", i=4),
                in_=psb[:].rearrange("g_ (i e) -> g_ i e", e=P))

        with ExitStack() as ectx:
            _edge_node_phases(ectx, tc, nc, cfg, cfg.L2, d, (t2_0, t2_1),
                              "2", w_e2, consts, node2)
        n2ctx.close()

        # ---------------- pool reduce + head ----------------
        with ExitStack() as hctx:
            hp = hctx.enter_context(tc.tile_pool(name="hd", bufs=1))
            hps = hctx.enter_context(tc.tile_pool(name="hd_ps", bufs=2,
                                                  space="PSUM"))
            pix = hp.tile([P, cfg.n_pool_idx // 16], I16)
            nc.sync.dma_start(pix[:], d["pool_idx"])
            NPB = cfg.n_pool_idx // P
            gpo = hp.tile([P, NPB * P], F32)
            nc.gpsimd.dma_gather(
                out_ap=gpo[:].rearrange("p (k e) -> p k e", e=P),
                in_ap=parts[:], idxs_ap=pix[:],
                num_idxs=cfg.n_pool_idx, num_idxs_reg=cfg.n_pool_idx,
                elem_size=P, single_packet=False)
            GB = GSP // P
            v = gpo[:].rearrange("p (q b e) -> p q b e", q=PG, b=GB)
            pooled = hp.tile([P, GB * P], F32)
            pv = pooled[:].rearrange("p (b e) -> p b e", b=GB)
            if PG == 1:
                nc.vector.tensor_copy(out=pv, in_=v[:, 0])
            else:
                nc.vector.tensor_add(out=pv, in0=v[:, 0], in1=v[:, 1])
                for q in range(2, PG):
                    nc.vector.tensor_add(out=pv, in0=pv, in1=v[:, q])
            cntg = hp.tile([P, GB], F32)
            nc.sync.dma_start(cntg[:], d["cnt_gm"])
            invc = hp.tile([P, GB], F32)
            nc.vector.reciprocal(invc[:], cntg[:])
            for b in range(GB):
                nc.vector.tensor_tensor(
                    out=pooled[:, b * P:(b + 1) * P],
                    in0=pooled[:, b * P:(b + 1) * P],
                    in1=invc[:, b:b + 1].to_broadcast([P, P]), op=ALU.mult)
            embT = hp.tile([P, GSP], BF16)
            for b in range(GB):
                tps = hps.tile([P, P], F32, tag="hd")
                nc.tensor.transpose(out=tps[:], in_=pooled[:, b * P:(b + 1) * P],
                                    identity=ident[:])
                nc.scalar.activation(out=embT[:, b * P:(b + 1) * P], in_=tps[:],
                                     func=AF.Identity, bias=b22[:])
            usrT = hp.tile([USR, GSP], BF16)
            nc.sync.dma_start(usrT[:], d["usrT"])
            hw = {nm: hp.tile(d[nm].shape,
                              F32 if nm.startswith("hb") else BF16,
                              name=f"t_{nm}")
                  for nm in ("hw1a", "hw1b", "hb1", "hw2", "hb2", "hw3", "hb3",
                             "hw4", "hb4", "hw5", "hb5")}
            for nm, t in hw.items():
                nc.sync.dma_start(t[:], d[nm])
            z1h = hps.tile([128, GSP], F32, tag="hd")
            nc.tensor.matmul(out=z1h[:], lhsT=hw["hw1a"][:], rhs=embT[:],
                             start=True, stop=False)
            nc.tensor.matmul(out=z1h[:], lhsT=hw["hw1b"][:], rhs=usrT[:],
                             start=False, stop=True)
            z1 = hp.tile([128, GSP], BF16)
            nc.scalar.activation(out=z1[:], in_=z1h[:], func=AF.Relu, bias=hw["hb1"][:])
            z2p = hps.tile([64, GSP], F32, tag="hd")
            nc.tensor.matmul(out=z2p[:], lhsT=hw["hw2"][:], rhs=z1[:], start=True, stop=True)
            z2 = hp.tile([64, GSP], BF16)
            nc.scalar.activation(out=z2[:], in_=z2p[:], func=AF.Relu, bias=hw["hb2"][:])
            z3p = hps.tile([32, GSP], F32, tag="hd")
            nc.tensor.matmul(out=z3p[:], lhsT=hw["hw3"][:], rhs=z2[:], start=True, stop=True)
            z3 = hp.tile([32, GSP], BF16)
            nc.scalar.activation(out=z3[:], in_=z3p[:], func=AF.Relu, bias=hw["hb3"][:])
            z4p = hps.tile([16, GSP], F32, tag="hd")
            nc.tensor.matmul(out=z4p[:], lhsT=hw["hw4"][:], rhs=z3[:], start=True, stop=True)
            z4 = hp.tile([16, GSP], BF16)
            nc.scalar.activation(out=z4[:], in_=z4p[:], func=AF.Relu, bias=hw["hb4"][:])
            z5p = hps.tile([1, GSP], F32, tag="hd")
            nc.tensor.matmul(out=z5p[:], lhsT=hw["hw5"][:], rhs=z4[:], start=True, stop=True)
            z5 = hp.tile([1, GSP], F32)
            nc.scalar.activation(out=z5[:], in_=z5p[:], func=AF.Identity, bias=hw["hb5"][:])
            nc.sync.dma_start(out=yT, in_=z5[:])

    nc.compile()
    return nc


def _make_in_maps(cfg, per_core, inputs, relids):
    f32 = lambda a: np.ascontiguousarray(np.asarray(a, np.float32))
    bf = lambda a: np.ascontiguousarray(np.asarray(a, np.float32).astype(bfloat16))
    x = f32(inputs["x"])
    usr = f32(inputs["usr"])
    tabs = _gather_tables(cfg, per_core, x)
    w_e1 = bf(np.vstack([np.asarray(inputs["e1_w"], np.float32),
                         np.asarray(inputs["e1_b"], np.float32)[None, :]]))
    w_e2 = bf(np.vstack([np.asarray(inputs["e2_w"], np.float32),
                         np.asarray(inputs["e2_b"], np.float32)[None, :]]))
    w12b = bf(np.vstack([np.asarray(inputs["n1_w2"], np.float32),
                         np.asarray(inputs["n1_b2"], np.float32)[None, :]]))
    iota = np.tile(np.arange(P, dtype=np.float32), (P, 1)).astype(bfloat16)
    NT = cfg.NT
    in_maps = []
    for c, pc in enumerate(per_core):
        usrT = np.zeros((USR, cfg.GSP), bfloat16)
        usrT[:, :cfg.GS] = usr[c * cfg.GS:(c + 1) * cfg.GS].T.astype(bfloat16)
        m = dict(
            tab0=tabs[c][0], tab1=tabs[c][1],
            xT=pc["xT"], sg1=_wrap16(pc["sg1"]), sg2=_wrap16(pc["sg2"]),
            w_e1=w_e1, w_e2=w_e2,
            w11=bf(inputs["n1_w1"]), b11=f32(inputs["n1_b1"])[:, None],
            w12=bf(inputs["n1_w2"]), b12=f32(inputs["n1_b2"])[:, None],
            w12b=w12b,
            w21=bf(inputs["n2_w1"]), b21=f32(inputs["n2_b1"])[:, None],
            w22=bf(inputs["n2_w2"]), b22=f32(inputs["n2_b2"])[:, None],
            iota=iota,
            relg=np.ascontiguousarray(pc["relg"].reshape(NT, P).T).astype(bfloat16),
            relids=relids.astype(bfloat16), pool_idx=_wrap16(pc["pool_idx"]),
            cnt_gm=pc["cnt_gm"], usrT=usrT,
            hw1a=bf(np.asarray(inputs["h1_w"])[:EMB]),
            hw1b=bf(np.asarray(inputs["h1_w"])[EMB:]),
            hb1=f32(inputs["h1_b"])[:, None],
            hw2=bf(inputs["h2_w"]), hb2=f32(inputs["h2_b"])[:, None],
            hw3=bf(inputs["h3_w"]), hb3=f32(inputs["h3_b"])[:, None],
            hw4=bf(inputs["h4_w"]), hb4=f32(inputs["h4_b"])[:, None],
            hw5=bf(inputs["h5_w"]), hb5=f32(inputs["h5_b"])[:, None])
        for ln in ("1", "2"):
            for s in (0, 1):
                gx, eaT, dr = pc[f"g{ln}"][s]
                m[f"gidx{ln}{s}"] = _wrap16(gx)
                m[f"eaT{ln}{s}"] = np.ascontiguousarray(eaT)
                m[f"dstrel{ln}{s}"] = np.ascontiguousarray(dr)
        in_maps.append(m)
    return in_maps


def kernel(**inputs):
    cfg, gb, per_core, relids = _preprocess(
        np.asarray(inputs["x"], np.float32), inputs["edge_index"],
        np.asarray(inputs["edge_attr"], np.float32), inputs["batch"])
    nc = _build(cfg)
    in_maps = _make_in_maps(cfg, per_core, inputs, relids)
    res = bass_utils.run_bass_kernel_spmd(nc, in_maps, core_ids=list(range(C)))
    out = np.concatenate([res.results[c]["yT"][0, :cfg.GS] for c in range(C)])
    kernel._last = res
    return out[:, None].astype(np.float32)


# revision 27
# speedup vs baseline: 1.1372x; 1.1372x over previous
"""Trainium2 Bass kernel for nn_DockingTimeModel (2-layer GINE GNN + mean-pool
+ MLP head), single merged SPMD launch on 8 NeuronCores.

v2 design: data-parallel over graphs (core c owns graphs [512c, 512(c+1)) and
their contiguous node range; edges live on the dst-owner core). Edges are
host-sorted by dst and packed into 128-edge chunks grouped per 128-node tile
(chunk counts maxed over cores so the module is SPMD-uniform). Per chunk:
dma_gather x[src] rows (256B) from compacted int16-indexed tables, edge linear
on PE (bias folded via ones-row, bf16), msg = relu(xg + lin) -> bf16, and the
segment-sum runs ON PE: aggT[64f, 128n] += msg[128e,64f].T @ S[128e,128n]
where S = is_equal(dstrel, iota) is built on DVE. No scatter DMA, no HBM
accumulator, aggregate lands feat-major in PSUM. Node MLPs consume it
directly; h1 is produced both feat-major (SBUF-resident bf16 for layer 2) and
node-major (DRAM for the AllToAll pre-gather). One AllToAll exchanges the h1
rows each core needs. Mean-pool via per-tile indicator matmuls; b22 is added
post-mean (linear). Head MLP on-chip; output [1, 512] per core.
"""
import sys

sys.path.insert(0, "/opt/trn_rl_repo")

import math
from contextlib import ExitStack
from dataclasses import dataclass, field

import numpy as np
from ml_dtypes import bfloat16

from concourse import bacc, bass, mybir, tile
from concourse import bass_utils
from concourse.masks import make_identity

F32 = mybir.dt.float32
BF16 = mybir.dt.bfloat16
I16 = mybir.dt.int16
AF = mybir.ActivationFunctionType
ALU = mybir.AluOpType

C = 8
P = 128
ND = 64
ED = 16
EMB = 128
USR = 12
EAROWS = ED + 1          # edge-attr rows + ones row (bias)
TAB0 = 32768
NCH = 512                # node-MLP group size
CPD = 16                 # gather chunks per DMA window


def _wrap16(idx):
    L = len(idx)
    assert L % 16 == 0
    a = np.asarray(idx, np.int16).reshape(L // 16, 16).T
    return np.tile(a, (8, 1))


@dataclass
class Layer:
    """Static chunk structure for one layer's edge phase (uniform across
    cores). k[t, s] chunks of 128 edges for node-tile t from stream s
    (s = table id). pos[t, s] = chunk offset within stream s."""
    k: np.ndarray                 # [NT, 2]
    pos: np.ndarray               # [NT, 2]
    nch: tuple                    # chunks per stream
    TAB1: int = 0                 # rows in table 1

    @property
    def slots(self):
        return (self.nch[0] * P, self.nch[1] * P)


@dataclass
class CFG:
    TAB1: int
    B: int
    N_SH: int
    L1: Layer
    L2: Layer
    B1: int = 0
    B2: int = 0
    H: int = 0
    GS: int = 512
    GSP: int = 512
    GW: int = 8
    PG: int = 2
    n_pool_idx: int = 0

    @property
    def NT(self):
        return self.N_SH // P


def _mk_layer(tb_all, tile_all, NT):
    """Uniform chunk structure: k[t, s] = max over cores of
    ceil(count(tile==t, tb==s) / 128)."""
    k = np.zeros((NT, 2), np.int64)
    for tb, tl in zip(tb_all, tile_all):
        for s in (0, 1):
            cnt = np.bincount(tl[tb == s], minlength=NT)
            k[:, s] = np.maximum(k[:, s], (cnt + P - 1) // P)
    dead = k.sum(1) == 0
    k[dead, 0] = 1
    pos = np.zeros((NT, 2), np.int64)
    pos[:, 0] = np.cumsum(k[:, 0]) - k[:, 0]
    pos[:, 1] = np.cumsum(k[:, 1]) - k[:, 1]
    return Layer(k=k, pos=pos, nch=(int(k[:, 0].sum()), int(k[:, 1].sum())))


def _fill_layer(lay, tb, tl, idx_local, dloc, ea, NT):
    """Place one core's edges into the two slot streams. Returns per stream:
    (gidx int16, eaT bf16 [EAROWS, slots], dstrel f32 [slots])."""
    out = []
    for s in (0, 1):
        slots = lay.nch[s] * P
        gidx = np.zeros(slots, np.int16)
        eaT = np.zeros((EAROWS, slots), np.float32)
        dstrel = np.full(slots, 255.0, np.float32)
        sel = np.nonzero(tb == s)[0]
        if len(sel):
            tls = tl[sel]
            order = sel[np.argsort(tls, kind="stable")]
            tls = tl[order]
            starts = np.searchsorted(tls, np.arange(NT))
            ends = np.searchsorted(tls, np.arange(NT), side="right")
            for t in range(NT):
                e = order[starts[t]:ends[t]]
                if not len(e):
                    continue
                o0 = lay.pos[t, s] * P
                assert len(e) <= lay.k[t, s] * P
                gidx[o0:o0 + len(e)] = idx_local[e].astype(np.int16)
                eaT[:ED, o0:o0 + len(e)] = ea[e].T
                eaT[ED, o0:o0 + len(e)] = 1.0
                dstrel[o0:o0 + len(e)] = (dloc[e] - t * P).astype(np.float32)
        out.append((gidx, eaT.astype(bfloat16),
                    dstrel.reshape(-1, P).T.astype(bfloat16)))
    return out


def _preprocess(x, edge_index, edge_attr, batch, G=4096):
    src = np.asarray(edge_index[0], np.int64)
    dst = np.asarray(edge_index[1], np.int64)
    batch = np.asarray(batch, np.int64)
    ea = np.asarray(edge_attr, np.float32)
    GS = G // C
    gb = np.searchsorted(batch, np.arange(0, G + 1, GS))
    ncnt = np.diff(gb)
    N_SH = int(math.ceil(ncnt.max() / NCH) * NCH)
    NT = N_SH // P
    owner = np.searchsorted(gb, dst, side="right") - 1

    cores = []
    for c in range(C):
        em = np.nonzero(owner == c)[0]
        s_c, d_c = src[em], dst[em] - gb[c]
        o = np.argsort(d_c, kind="stable")
        s_c, d_c, ea_c = s_c[o], d_c[o], ea[em[o]]
        uniq, inv = np.unique(s_c, return_inverse=True)
        cores.append(dict(s=s_c, dloc=d_c, ea=ea_c, uniq=uniq, inv=inv,
                          tile=d_c // P, n_c=ncnt[c]))
    max_u = max(len(pc["uniq"]) for pc in cores)
    TAB1 = int(math.ceil(max(max_u - TAB0, P) / P) * P)

    # layer-1 structure: table split by uniq position
    tb1 = [(pc["inv"] >= TAB0).astype(np.int64) for pc in cores]
    L1 = _mk_layer(tb1, [pc["tile"] for pc in cores], NT)
    L1.TAB1 = TAB1

    # a2a in two phases split at owner-local node id H: phase tables are
    # each a single <=32768-row gather table and collective #1 overlaps the
    # tail of layer 1.
    for c in range(C):
        own = np.searchsorted(gb, cores[c]["uniq"], side="right") - 1
        cores[c]["uniq_owner"] = own
        cores[c]["uniq_lid"] = cores[c]["uniq"] - gb[own]
    H = (int(N_SH * 0.75) // NCH) * NCH
    while H > 0:
        need1 = np.zeros((C, C), np.int64)
        for c in range(C):
            own, lid = cores[c]["uniq_owner"], cores[c]["uniq_lid"]
            for o in range(C):
                need1[c, o] = int(((own == o) & (lid < H)).sum())
        B1 = int(math.ceil((need1.max() + 1) / P) * P)
        if C * B1 <= TAB0:
            break
        H -= NCH
    assert H > 0
    need2 = np.zeros((C, C), np.int64)
    for c in range(C):
        own, lid = cores[c]["uniq_owner"], cores[c]["uniq_lid"]
        for o in range(C):
            need2[c, o] = int(((own == o) & (lid >= H)).sum())
    B2 = int(math.ceil((need2.max() + 1) / P) * P)
    assert C * B2 <= TAB0
    for c in range(C):
        own, lid = cores[c]["uniq_owner"], cores[c]["uniq_lid"]
        ph = (lid >= H).astype(np.int64)
        r = np.zeros(len(own), np.int64)
        for o in range(C):
            for p_ in (0, 1):
                m = (own == o) & (ph == p_)
                r[m] = np.arange(m.sum())
        posu = np.where(ph == 0, own * B1 + r, own * B2 + r)
        cores[c]["pos2"] = posu[cores[c]["inv"]]
        cores[c]["ph2"] = ph[cores[c]["inv"]]

    tb2 = [pc["ph2"] for pc in cores]
    L2 = _mk_layer(tb2, [pc["tile"] for pc in cores], NT)
    B = B1

    GSP = GS
    cfg = CFG(TAB1=TAB1, B=B, N_SH=N_SH, L1=L1, L2=L2, GS=GS, GSP=GSP,
              B1=B1, B2=B2, H=H)

    per_core = []
    for c in range(C):
        pc = cores[c]
        g1 = _fill_layer(L1, tb1[c], pc["tile"], np.where(
            pc["inv"] < TAB0, pc["inv"], pc["inv"] - TAB0),
            pc["dloc"], pc["ea"], NT)
        g2 = _fill_layer(L2, tb2[c], pc["tile"], pc["pos2"],
                         pc["dloc"], pc["ea"], NT)

        n_c = pc["n_c"]
        xT = np.zeros((ND, N_SH), np.float32)
        xT[:, :n_c] = np.asarray(x)[gb[c]:gb[c + 1]].T

        # pooling structures (same scheme as v1)
        bl = batch[gb[c]:gb[c + 1]] - c * GS
        blp = np.full(N_SH, -1, np.int64)
        blp[:n_c] = bl
        tiles = blp.reshape(NT, P)
        g_first = np.array([t[t >= 0].min() if (t >= 0).any() else 0
                            for t in tiles])
        relg = np.where(blp >= 0, blp - np.repeat(g_first, P), 255.0)
        cnt = np.bincount(bl, minlength=GS).astype(np.float32)
        gstart = np.searchsorted(bl, np.arange(GS))
        gend = np.searchsorted(bl, np.arange(GS), side="right")
        t_lo, t_hi = gstart // P, np.maximum(gend - 1, gstart) // P

        per_core.append(dict(
            g1=g1, g2=g2, xT=xT, uniq=pc["uniq"],
            uniq_owner=pc["uniq_owner"], n_c=n_c,
            relg=relg.astype(np.float32), g_first=g_first, cnt=cnt,
            t_lo=t_lo, t_hi=t_hi))

    # send-side gather indices, per phase (phase 2 rows offset by -H
    # because they gather from h1r2)
    for o in range(C):
        sg1 = np.zeros(C * B1, np.int16)
        sg2 = np.zeros(C * B2, np.int16)
        for c in range(C):
            m = per_core[c]["uniq_owner"] == o
            rows = per_core[c]["uniq"][m] - gb[o]
            r1 = rows[rows < H]
            r2 = rows[rows >= H] - H
            sg1[c * B1:c * B1 + len(r1)] = r1.astype(np.int16)
            sg2[c * B2:c * B2 + len(r2)] = r2.astype(np.int16)
        per_core[o]["sg1"] = sg1
        per_core[o]["sg2"] = sg2

    cfg.GW = int(max((pc["relg"][pc["relg"] != 255.0]).max() + 1
                     if (pc["relg"] != 255.0).any() else 1 for pc in per_core))
    cfg.PG = int(max((pc["t_hi"] - pc["t_lo"] + 1)[pc["cnt"] > 0].max()
                     if (pc["cnt"] > 0).any() else 1 for pc in per_core))
    cfg.n_pool_idx = int(math.ceil(cfg.PG * cfg.GSP / P) * P)

    ZPAD = NT * cfg.GW
    for pc in per_core:
        pidx = np.full(cfg.n_pool_idx, ZPAD, np.int16)
        for g in range(GS):
            if pc["cnt"][g] <= 0:
                continue
            for p_, t in enumerate(range(pc["t_lo"][g], pc["t_hi"][g] + 1)):
                rel = g - pc["g_first"][t]
                pidx[p_ * cfg.GSP + g] = t * cfg.GW + rel
        pc["pool_idx"] = pidx
        pc["cnt_gm"] = np.maximum(
            np.pad(pc["cnt"], (0, cfg.GSP - GS)), 1.0
        ).reshape(cfg.GSP // P, P).T.astype(np.float32)

    relids = np.tile(np.arange(cfg.GW, dtype=np.float32), (P, 1))
    return cfg, gb, per_core, relids


def _gather_tables(cfg, per_core, x):
    out = []
    for pc in per_core:
        uniq = pc["uniq"]
        t0 = np.zeros((TAB0, ND), np.float32)
        t1 = np.zeros((cfg.TAB1, ND), np.float32)
        n0 = min(len(uniq), TAB0)
        t0[:n0] = x[uniq[:n0]]
        if len(uniq) > TAB0:
            t1[:len(uniq) - TAB0] = x[uniq[TAB0:]]
        out.append((t0, t1))
    return out


def _edge_node_phases(ctx, tc, nc, cfg, lay, d, tabs, names, w_e, consts,
                      node_cb):
    """Edge phase (gather + lin + msg + S-matmul aggregation) interleaved with
    per-512-node-group node_cb(g, agg_tiles[4])."""
    iota = consts["iota"]
    NT = cfg.NT

    gp = ctx.enter_context(tc.tile_pool(name=f"g{names[0]}", bufs=1))
    gidx = [gp.tile([P, lay.nch[s] * 8], I16, name=f"gx{names[0]}{s}")
            for s in (0, 1)]
    drl = [gp.tile([P, lay.nch[s]], BF16, name=f"dr{names[0]}{s}")
           for s in (0, 1)]
    for s in (0, 1):
        nc.sync.dma_start(gidx[s][:], d[f"gidx{names[0]}{s}"])
        nc.sync.dma_start(drl[s][:], d[f"dstrel{names[0]}{s}"])

    SB = 8                       # lin chunks per PSUM bank
    xp = ctx.enter_context(tc.tile_pool(name=f"xg{names[0]}", bufs=3))
    ep = ctx.enter_context(tc.tile_pool(name=f"ea{names[0]}", bufs=3))
    mp = ctx.enter_context(tc.tile_pool(name=f"ms{names[0]}", bufs=3))
    sp = ctx.enter_context(tc.tile_pool(name=f"S{names[0]}", bufs=3))
    lp = ctx.enter_context(tc.tile_pool(name=f"lin{names[0]}", bufs=2,
                                        space="PSUM"))
    ap = ctx.enter_context(tc.tile_pool(name=f"agg{names[0]}", bufs=1,
                                        space="PSUM"))

    state = [dict(win=-1), dict(win=-1)]

    def window(s, j):
        w = j // CPD
        st = state[s]
        if st["win"] != w:
            n = min(CPD, lay.nch[s] - w * CPD)
            xg = xp.tile([P, CPD * ND], F32, tag=f"xg{s}")
            nc.gpsimd.dma_gather(
                out_ap=xg[:, :n * ND].rearrange("p (k e) -> p k e", e=ND),
                in_ap=tabs[s], idxs_ap=gidx[s][:, w * CPD * 8:(w * CPD + n) * 8],
                num_idxs=n * P, num_idxs_reg=n * P, elem_size=ND,
                single_packet=False)
            eat = ep.tile([EAROWS, CPD * P], BF16, tag=f"ea{s}")
            nc.sync.dma_start(
                eat[:, :n * P],
                d[f"eaT{names[0]}{s}"][:, w * CPD * P:(w * CPD + n) * P])
            # edge linear into PSUM banks of SB chunks, then one add+relu
            # per bank -> bf16 msg; one is_equal for the whole window's S
            msg = mp.tile([P, CPD * ND], BF16, tag=f"ms{s}")
            for b0 in range(0, n, SB):
                nb = min(SB, n - b0)
                linb = lp.tile([P, SB * ND], F32, tag=f"lb{s}")
                for jj in range(nb):
                    nc.tensor.matmul(
                        out=linb[:, jj * ND:(jj + 1) * ND],
                        lhsT=eat[:, (b0 + jj) * P:(b0 + jj + 1) * P],
                        rhs=w_e[:], start=True, stop=True)
                sl = slice(b0 * ND, (b0 + nb) * ND)
                nc.vector.tensor_add(out=msg[:, sl], in0=xg[:, sl],
                                     in1=linb[:, :nb * ND])
                nc.scalar.activation(out=msg[:, sl], in_=msg[:, sl],
                                     func=AF.Relu)
            S = sp.tile([P, CPD * P], BF16, tag=f"S{s}")
            nc.vector.tensor_tensor(
                out=S[:, :n * P].rearrange("p (j c) -> p j c", c=P),
                in0=drl[s][:, w * CPD:w * CPD + n]
                .unsqueeze(2).broadcast_to([P, n, P]),
                in1=iota[:].unsqueeze(1).broadcast_to([P, n, P]),
                op=ALU.is_equal)
            st.update(win=w, msg=msg, S=S)
        return st["msg"], st["S"], j - w * CPD

    agg = None
    for t in range(NT):
        if t % 4 == 0:
            agg = ap.tile([ND, 4 * P], F32, tag=f"agg{(t // 4) % 2}")
        i4 = t % 4
        nk = int(lay.k[t, 0] + lay.k[t, 1])
        ci = 0
        for s in (0, 1):
            for i in range(int(lay.k[t, s])):
                j = int(lay.pos[t, s]) + i
                msg, S, jj = window(s, j)
                nc.tensor.matmul(out=agg[:, i4 * P:(i4 + 1) * P],
                                 lhsT=msg[:, jj * ND:(jj + 1) * ND],
                                 rhs=S[:, jj * P:(jj + 1) * P],
                                 start=(ci == 0), stop=(ci == nk - 1))
                ci += 1
        if t % 4 == 3:
            node_cb(t // 4, agg)


def _build(cfg):
    nc = bacc.Bacc("TRN2", target_bir_lowering=False, debug=False,
                   num_devices=C)
    d = {}

    def inp(name, shape, dt=F32):
        d[name] = nc.dram_tensor(name, shape, dt, kind="ExternalInput").ap()

    inp("tab0", [TAB0, ND]); inp("tab1", [cfg.TAB1, ND])
    for ln, lay in (("1", cfg.L1), ("2", cfg.L2)):
        for s in (0, 1):
            inp(f"gidx{ln}{s}", [P, lay.nch[s] * 8], I16)
            inp(f"eaT{ln}{s}", [EAROWS, lay.nch[s] * P], BF16)
            inp(f"dstrel{ln}{s}", [P, lay.nch[s]], BF16)
    inp("xT", [ND, cfg.N_SH])
    inp("sg1", [P, C * cfg.B1 // 16], I16)
    inp("sg2", [P, C * cfg.B2 // 16], I16)
    inp("w_e1", [EAROWS, ND], BF16); inp("w_e2", [EAROWS, ND], BF16)
    inp("w11", [ND, ND], BF16); inp("b11", [ND, 1])
    inp("w12", [ND, ND], BF16); inp("b12", [ND, 1])
    inp("w12b", [ND + 1, ND], BF16)
    inp("w21", [ND, EMB], BF16); inp("b21", [EMB, 1])
    inp("w22", [EMB, EMB], BF16); inp("b22", [EMB, 1])
    inp("iota", [P, P], BF16)
    inp("relg", [P, cfg.NT], BF16); inp("relids", [P, cfg.GW], BF16)
    inp("pool_idx", [P, cfg.n_pool_idx // 16], I16)
    inp("cnt_gm", [P, cfg.GSP // P]); inp("usrT", [USR, cfg.GSP], BF16)
    for nm, shp in (("hw1a", [EMB, 128]), ("hw1b", [USR, 128]), ("hb1", [128, 1]),
                    ("hw2", [128, 64]), ("hb2", [64, 1]), ("hw3", [64, 32]),
                    ("hb3", [32, 1]), ("hw4", [32, 16]), ("hb4", [16, 1]),
                    ("hw5", [16, 1]), ("hb5", [1, 1])):
        inp(nm, shp, F32 if nm.startswith("hb") else BF16)
    yT = nc.dram_tensor("yT", [1, cfg.GSP], F32, kind="ExternalOutput").ap()

    GW, PG, NT, GSP, B = cfg.GW, cfg.PG, cfg.NT, cfg.GSP, cfg.B
    NROW = NT * GW + P
    NG = cfg.N_SH // NCH

    with tile.TileContext(nc) as tc, ExitStack() as ctx:
        const = ctx.enter_context(tc.tile_pool(name="const", bufs=1))

        def ld(name, shape, dt=F32):
            t = const.tile(shape, dt, name=f"c_{name}")
            nc.sync.dma_start(t[:], d[name])
            return t

        w_e1 = ld("w_e1", [EAROWS, ND], BF16)
        w_e2 = ld("w_e2", [EAROWS, ND], BF16)
        w11 = ld("w11", [ND, ND], BF16); b11 = ld("b11", [ND, 1])
        w12 = ld("w12", [ND, ND], BF16); b12 = ld("b12", [ND, 1])
        w12b = ld("w12b", [ND + 1, ND], BF16)
        w21 = ld("w21", [ND, EMB], BF16); b21 = ld("b21", [EMB, 1])
        w22 = ld("w22", [EMB, EMB], BF16); b22 = ld("b22", [EMB, 1])
        iota = ld("iota", [P, P], BF16)
        relg = ld("relg", [P, NT], BF16)
        relids = ld("relids", [P, GW], BF16)
        ident = const.tile([P, P], F32, name="ident")
        make_identity(nc, ident[:])
        zt = const.tile([P, P], F32, name="zt")
        nc.vector.memset(zt[:], 0.0)
        h1T = const.tile([ND, cfg.N_SH], BF16, name="h1T")

        dram = ctx.enter_context(tc.tile_pool(name="dram", bufs=1, space="DRAM"))
        B1, B2, H = cfg.B1, cfg.B2, cfg.H
        GH = H // NCH
        h1r1 = dram.tile([H, ND], F32)
        h1r2 = dram.tile([cfg.N_SH - H, ND], F32)
        a2a_in1 = dram.tile([C * B1, ND], F32)
        a2a_out1 = dram.tile([C * B1, ND], F32)
        a2a_in2 = dram.tile([C * B2, ND], F32)
        a2a_out2 = dram.tile([C * B2, ND], F32)
        parts = dram.tile([NROW, P], F32)
        nc.sync.dma_start(
            out=parts[NT * GW:NT * GW + P, :].rearrange("(p r) e -> p (r e)", p=P),
            in_=zt[:, :P])

        consts = dict(iota=iota)

        agp = ctx.enter_context(tc.tile_pool(name="a2a", bufs=2))
        sgp = ctx.enter_context(tc.tile_pool(name="a2as", bufs=2))

        def a2a_phase(src_t, sgname, Bp, ain, aout):
            for dest in range(C):
                six = sgp.tile([P, Bp // 16], I16, tag="six")
                nc.sync.dma_start(
                    six[:], d[sgname][:, dest * Bp // 16:(dest + 1) * Bp // 16])
                gt = agp.tile([P, Bp // P * ND], F32, tag="gt")
                nc.gpsimd.dma_gather(
                    out_ap=gt[:].rearrange("p (k e) -> p k e", e=ND),
                    in_ap=src_t[:], idxs_ap=six[:],
                    num_idxs=Bp, num_idxs_reg=Bp, elem_size=ND,
                    single_packet=False)
                nc.sync.dma_start(
                    out=ain[dest * Bp:(dest + 1) * Bp, :]
                    .rearrange("(k p) e -> p k e", p=P),
                    in_=gt[:].rearrange("p (k e) -> p k e", e=ND))
            nc.gpsimd.collective_compute(
                "AllToAll", mybir.AluOpType.bypass,
                replica_groups=[list(range(C))],
                ins=[ain[:].opt()], outs=[aout[:].opt()])

        # ---------------- layer 1 ----------------
        n1ctx = ExitStack()
        xtp = n1ctx.enter_context(tc.tile_pool(name="n1_xt", bufs=2))
        htp = n1ctx.enter_context(tc.tile_pool(name="n1_ht", bufs=2))
        z1p = n1ctx.enter_context(tc.tile_pool(name="n1_z1", bufs=2))
        rwp = n1ctx.enter_context(tc.tile_pool(name="n1_rw", bufs=3))
        nps = n1ctx.enter_context(tc.tile_pool(name="n1_ps", bufs=1,
                                               space="PSUM"))

        def node1(g, agg):
            sl = slice(g * NCH, (g + 1) * NCH)
            xTb = xtp.tile([ND, NCH], F32, tag="xt")
            nc.sync.dma_start(xTb[:], d["xT"][:, sl])
            hT = htp.tile([ND, NCH], BF16, tag="ht")
            nc.vector.tensor_add(out=hT[:], in0=xTb[:], in1=agg[:])
            z1ps = nps.tile([ND, NCH], F32, tag="nps")
            nc.tensor.matmul(out=z1ps[:], lhsT=w11[:], rhs=hT[:],
                             start=True, stop=True)
            z1x = z1p.tile([ND + 1, NCH], BF16, tag="z1")
            nc.scalar.activation(out=z1x[:ND, :], in_=z1ps[:], func=AF.Relu,
                                 bias=b11[:])
            nc.vector.memset(z1x[ND:ND + 1, :], 1.0)
            hps = nps.tile([ND, NCH], F32, tag="nps")
            nc.tensor.matmul(out=hps[:], lhsT=w12[:], rhs=z1x[:ND, :],
                             start=True, stop=True)
            nc.scalar.activation(out=h1T[:, sl], in_=hps[:], func=AF.Relu,
                                 bias=b12[:])
            nmps = nps.tile([P, 4 * ND], F32, tag="nm")
            for i in range(4):
                nc.tensor.matmul(out=nmps[:, i * ND:(i + 1) * ND],
                                 lhsT=z1x[:, i * P:(i + 1) * P],
                                 rhs=w12b[:], start=True, stop=True)
            rows = rwp.tile([P, 4 * ND], F32, tag="rw")
            nc.scalar.activation(out=rows[:], in_=nmps[:], func=AF.Relu)
            h1r_t, r0 = (h1r1, g * NCH) if g < GH else (h1r2, (g - GH) * NCH)
            nc.sync.dma_start(
                out=h1r_t[r0:r0 + NCH, :]
                .rearrange("(i p) e -> p i e", p=P),
                in_=rows[:].rearrange("p (i e) -> p i e", e=ND))
            if g == GH - 1:
                a2a_phase(h1r1, "sg1", B1, a2a_in1, a2a_out1)

        with ExitStack() as ectx:
            _edge_node_phases(ectx, tc, nc, cfg, cfg.L1, d,
                              (d["tab0"], d["tab1"]), "1", w_e1, consts, node1)
        n1ctx.close()

        # ---------------- exchange phase 2 ----------------
        a2a_phase(h1r2, "sg2", B2, a2a_in2, a2a_out2)

        # ---------------- layer 2 ----------------
        t2_0 = a2a_out1[:]
        t2_1 = a2a_out2[:]

        n2ctx = ExitStack()
        h1fp = n2ctx.enter_context(tc.tile_pool(name="n2_h1", bufs=2))
        ht2p = n2ctx.enter_context(tc.tile_pool(name="n2_ht", bufs=2))
        zxp = n2ctx.enter_context(tc.tile_pool(name="n2_zx", bufs=2))
        emp = n2ctx.enter_context(tc.tile_pool(name="n2_em", bufs=3))
        s4p = n2ctx.enter_context(tc.tile_pool(name="n2_s4", bufs=2))
        psp = n2ctx.enter_context(tc.tile_pool(name="n2_pl", bufs=3))
        nps2 = n2ctx.enter_context(tc.tile_pool(name="n2_ps", bufs=1,
                                                space="PSUM"))

        def node2(g, agg):
            sl = slice(g * NCH, (g + 1) * NCH)
            h1f = h1fp.tile([ND, NCH], F32, tag="h1f")
            nc.vector.tensor_copy(out=h1f[:], in_=h1T[:, sl])
            hT2 = ht2p.tile([ND, NCH], BF16, tag="ht2")
            nc.vector.tensor_add(out=hT2[:], in0=h1f[:], in1=agg[:])
            zps = nps2.tile([EMB, NCH], F32, tag="nps")
            nc.tensor.matmul(out=zps[:], lhsT=w21[:], rhs=hT2[:],
                             start=True, stop=True)
            zx = zxp.tile([EMB, NCH], BF16, tag="zx")
            nc.scalar.activation(out=zx[:], in_=zps[:], func=AF.Relu,
                                 bias=b21[:])
            S4 = s4p.tile([P, 4 * GW], BF16, tag="S4")
            nc.vector.tensor_tensor(
                out=S4[:].rearrange("p (j g) -> p j g", g=GW),
                in0=relg[:, g * 4:(g + 1) * 4]
                .unsqueeze(2).broadcast_to([P, 4, GW]),
                in1=relids[:].unsqueeze(1).broadcast_to([P, 4, GW]),
                op=ALU.is_equal)
            emps = nps2.tile([P, 4 * EMB], F32, tag="nm")
            for i in range(4):
                nc.tensor.matmul(out=emps[:, i * EMB:(i + 1) * EMB],
                                 lhsT=zx[:, i * P:(i + 1) * P],
                                 rhs=w22[:], start=True, stop=True)
            emb_nm = emp.tile([P, 4 * EMB], BF16, tag="em")
            nc.vector.tensor_copy(out=emb_nm[:], in_=emps[:])
            pps = nps2.tile([GW, 4 * P], F32, tag="nm")
            for i in range(4):
                nc.tensor.matmul(out=pps[:, i * P:(i + 1) * P],
                                 lhsT=S4[:, i * GW:(i + 1) * GW],
                                 rhs=emb_nm[:, i * EMB:(i + 1) * EMB],
                                 start=True, stop=True)
            psb = psp.tile([GW, 4 * P], F32, tag="pls")
            nc.vector.tensor_copy(out=psb[:], in_=pps[:])
            nc.sync.dma_start(
                out=parts[g * 4 * GW:(g + 1) * 4 * GW, :]
                .rearrange("(i g_) e -> g_ i e", i=4),
                in_=psb[:].rearrange("g_ (i e) -> g_ i e", e=P))

        with ExitStack() as ectx:
            _edge_node_phases(ectx, tc, nc, cfg, cfg.L2, d, (t2_0, t2_1),
                              "2", w_e2, consts, node2)
        n2ctx.close()

        # ---------------- pool reduce + head ----------------
        with ExitStack() as hctx:
            hp = hctx.enter_context(tc.tile_pool(name="hd", bufs=1))
            hps = hctx.enter_context(tc.tile_pool(name="hd_ps", bufs=2,
                                                  space="PSUM"))
            pix = hp.tile([P, cfg.n_pool_idx // 16], I16)
            nc.sync.dma_start(pix[:], d["pool_idx"])
            NPB = cfg.n_pool_idx // P
            gpo = hp.tile([P, NPB * P], F32)
            nc.gpsimd.dma_gather(
                out_ap=gpo[:].rearrange("p (k e) -> p k e", e=P),
                in_ap=parts[:], idxs_ap=pix[:],
                num_idxs=cfg.n_pool_idx, num_idxs_reg=cfg.n_pool_idx,
                elem_size=P, single_packet=False)
            GB = GSP // P
            v = gpo[:].rearrange("p (q b e) -> p q b e", q=PG, b=GB)
            pooled = hp.tile([P, GB * P], F32)
            pv = pooled[:].rearrange("p (b e) -> p b e", b=GB)
            if PG == 1:
                nc.vector.tensor_copy(out=pv, in_=v[:, 0])
            else:
                nc.vector.tensor_add(out=pv, in0=v[:, 0], in1=v[:, 1])
                for q in range(2, PG):
                    nc.vector.tensor_add(out=pv, in0=pv, in1=v[:, q])
            cntg = hp.tile([P, GB], F32)
            nc.sync.dma_start(cntg[:], d["cnt_gm"])
            invc = hp.tile([P, GB], F32)
            nc.vector.reciprocal(invc[:], cntg[:])
            for b in range(GB):
                nc.vector.tensor_tensor(
                    out=pooled[:, b * P:(b + 1) * P],
                    in0=pooled[:, b * P:(b + 1) * P],
                    in1=invc[:, b:b + 1].to_broadcast([P, P]), op=ALU.mult)
            embT = hp.tile([P, GSP], BF16)
            for b in range(GB):
                tps = hps.tile([P, P], F32, tag="hd")
                nc.tensor.transpose(out=tps[:], in_=pooled[:, b * P:(b + 1) * P],
                                    identity=ident[:])
                nc.scalar.activation(out=embT[:, b * P:(b + 1) * P], in_=tps[:],
                                     func=AF.Identity, bias=b22[:])
            usrT = hp.tile([USR, GSP], BF16)
            nc.sync.dma_start(usrT[:], d["usrT"])
            hw = {nm: hp.tile(d[nm].shape,
                              F32 if nm.startswith("hb") else BF16,
                              name=f"t_{nm}")
                  for nm in ("hw1a", "hw1b", "hb1", "hw2", "hb2", "hw3", "hb3",
                             "hw4", "hb4", "hw5", "hb5")}
            for nm, t in hw.items():
                nc.sync.dma_start(t[:], d[nm])
            z1h = hps.tile([128, GSP], F32, tag="hd")
            nc.tensor.matmul(out=z1h[:], lhsT=hw["hw1a"][:], rhs=embT[:],
                             start=True, stop=False)
            nc.tensor.matmul(out=z1h[:], lhsT=hw["hw1b"][:], rhs=usrT[:],
                             start=False, stop=True)
            z1 = hp.tile([128, GSP], BF16)
            nc.scalar.activation(out=z1[:], in_=z1h[:], func=AF.Relu, bias=hw["hb1"][:])
            z2p = hps.tile([64, GSP], F32, tag="hd")
            nc.tensor.matmul(out=z2p[:], lhsT=hw["hw2"][:], rhs=z1[:], start=True, stop=True)
            z2 = hp.tile([64, GSP], BF16)
            nc.scalar.activation(out=z2[:], in_=z2p[:], func=AF.Relu, bias=hw["hb2"][:])
            z3p = hps.tile([32, GSP], F32, tag="hd")
            nc.tensor.matmul(out=z3p[:], lhsT=hw["hw3"][:], rhs=z2[:], start=True, stop=True)
            z3 = hp.tile([32, GSP], BF16)
            nc.scalar.activation(out=z3[:], in_=z3p[:], func=AF.Relu, bias=hw["hb3"][:])
            z4p = hps.tile([16, GSP], F32, tag="hd")
            nc.tensor.matmul(out=z4p[:], lhsT=hw["hw4"][:], rhs=z3[:], start=True, stop=True)
            z4 = hp.tile([16, GSP], BF16)
            nc.scalar.activation(out=z4[:], in_=z4p[:], func=AF.Relu, bias=hw["hb4"][:])
            z5p = hps.tile([1, GSP], F32, tag="hd")
            nc.tensor.matmul(out=z5p[:], lhsT=hw["hw5"][:], rhs=z4[:], start=True, stop=True)
            z5 = hp.tile([1, GSP], F32)
            nc.scalar.activation(out=z5[:], in_=z5p[:], func=AF.Identity, bias=hw["hb5"][:])
            nc.sync.dma_start(out=yT, in_=z5[:])

    nc.compile()
    return nc


def _make_in_maps(cfg, per_core, inputs, relids):
    f32 = lambda a: np.ascontiguousarray(np.asarray(a, np.float32))
    bf = lambda a: np.ascontiguousarray(np.asarray(a, np.float32).astype(bfloat16))
    x = f32(inputs["x"])
    usr = f32(inputs["usr"])
    tabs = _gather_tables(cfg, per_core, x)
    w_e1 = bf(np.vstack([np.asarray(inputs["e1_w"], np.float32),
                         np.asarray(inputs["e1_b"], np.float32)[None, :]]))
    w_e2 = bf(np.vstack([np.asarray(inputs["e2_w"], np.float32),
                         np.asarray(inputs["e2_b"], np.float32)[None, :]]))
    w12b = bf(np.vstack([np.asarray(inputs["n1_w2"], np.float32),
                         np.asarray(inputs["n1_b2"], np.float32)[None, :]]))
    iota = np.tile(np.arange(P, dtype=np.float32), (P, 1)).astype(bfloat16)
    NT = cfg.NT
    in_maps = []
    for c, pc in enumerate(per_core):
        usrT = np.zeros((USR, cfg.GSP), bfloat16)
        usrT[:, :cfg.GS] = usr[c * cfg.GS:(c + 1) * cfg.GS].T.astype(bfloat16)
        m = dict(
            tab0=tabs[c][0], tab1=tabs[c][1],
            xT=pc["xT"], sg1=_wrap16(pc["sg1"]), sg2=_wrap16(pc["sg2"]),
            w_e1=w_e1, w_e2=w_e2,
            w11=bf(inputs["n1_w1"]), b11=f32(inputs["n1_b1"])[:, None],
            w12=bf(inputs["n1_w2"]), b12=f32(inputs["n1_b2"])[:, None],
            w12b=w12b,
            w21=bf(inputs["n2_w1"]), b21=f32(inputs["n2_b1"])[:, None],
            w22=bf(inputs["n2_w2"]), b22=f32(inputs["n2_b2"])[:, None],
            iota=iota,
            relg=np.ascontiguousarray(pc["relg"].reshape(NT, P).T).astype(bfloat16),
            relids=relids.astype(bfloat16), pool_idx=_wrap16(pc["pool_idx"]),
            cnt_gm=pc["cnt_gm"], usrT=usrT,
            hw1a=bf(np.asarray(inputs["h1_w"])[:EMB]),
            hw1b=bf(np.asarray(inputs["h1_w"])[EMB:]),
            hb1=f32(inputs["h1_b"])[:, None],
            hw2=bf(inputs["h2_w"]), hb2=f32(inputs["h2_b"])[:, None],
            hw3=bf(inputs["h3_w"]), hb3=f32(inputs["h3_b"])[:, None],
            hw4=bf(inputs["h4_w"]), hb4=f32(inputs["h4_b"])[:, None],
            hw5=bf(inputs["h5_w"]), hb5=f32(inputs["h5_b"])[:, None])
        for ln in ("1", "2"):
            for s in (0, 1):
                gx, eaT, dr = pc[f"g{ln}"][s]
                m[f"gidx{ln}{s}"] = _wrap16(gx)
                m[f"eaT{ln}{s}"] = np.ascontiguousarray(eaT)
                m[f"dstrel{ln}{s}"] = np.ascontiguousarray(dr)
        in_maps.append(m)
    return in_maps


def kernel(**inputs):
    cfg, gb, per_core, relids = _preprocess(
        np.asarray(inputs["x"], np.float32), inputs["edge_index"],
        np.asarray(inputs["edge_attr"], np.float32), inputs["batch"])
    nc = _build(cfg)
    in_maps = _make_in_maps(cfg, per_core, inputs, relids)
    res = bass_utils.run_bass_kernel_spmd(nc, in_maps, core_ids=list(range(C)))
    out = np.concatenate([res.results[c]["yT"][0, :cfg.GS] for c in range(C)])
    kernel._last = res
    return out[:, None].astype(np.float32)
